# revision 3
# baseline (speedup 1.0000x reference)
"""Trainium2 Bass kernel for nn_ModelRQuery_5806795784426.

Strategy (data-parallel over bags, 8 cores x 64 bags):
  - node_weight (cosine-sim softmax) is computed with the exact same eager
    jax ops as the reference, so the Huffman merge schedule derived from it
    is bit-faithful to the reference's argmin decisions on this backend.
  - The Huffman weight evolution is replayed on host (pure IEEE f32 adds on
    identical bits -> identical schedule), producing per-bag merge pairs in
    an append-only slot numbering (leaves 0..63, merge t -> slot 64+t).
  - The heavy part (63 sequential 2-layer MLP merge steps, batched over 64
    bags per core) runs on device: indirect-DMA gather of the two operand
    feature rows per bag, tanh, fc1, tanh, fc2, scatter to the new slot.
  - Final: scores = root_feat @ rel_emb.T, sigmoid.
"""

import numpy as np

NB = 64      # bags per core
NN = 64      # nodes (leaves) per bag
SL = 128     # slots per bag (64 leaves + 63 merges, padded)
D = 1024
NSTEP = NN - 1
CPAD = 64    # rel classes padded 53 -> 64
NCORES = 8

_PROG = {}


def _build_program():
    if "nc" in _PROG:
        return _PROG
    import concourse.bass as bass
    import concourse.bacc as bacc
    import concourse.tile as tile

    mybir = bass.mybir
    f32 = mybir.dt.float32
    f32r = mybir.dt.float32r
    i32 = mybir.dt.int32
    TANH = mybir.ActivationFunctionType.Tanh
    SIG = mybir.ActivationFunctionType.Sigmoid
    ADD = mybir.AluOpType.add

    nc = bacc.Bacc(None, target_bir_lowering=False)
    rep_d = nc.dram_tensor("rep", [NB * NN, D], f32, kind="ExternalInput")
    w1t_d = nc.dram_tensor("w1t", [2 * D, D], f32r, kind="ExternalInput")
    w2t_d = nc.dram_tensor("w2t", [D, D], f32r, kind="ExternalInput")
    relt_d = nc.dram_tensor("relt", [D, CPAD], f32, kind="ExternalInput")
    b1b_d = nc.dram_tensor("b1b", [NB, D], f32, kind="ExternalInput")
    b2b_d = nc.dram_tensor("b2b", [NB, D], f32, kind="ExternalInput")
    gidx_d = nc.dram_tensor("gidx", [NB, 2 * NSTEP], i32, kind="ExternalInput")
    ident_d = nc.dram_tensor("ident", [64, 64], f32, kind="ExternalInput")
    out_d = nc.dram_tensor("out", [NB, CPAD], f32, kind="ExternalOutput")
    feats_d = nc.dram_tensor("feats", [NB * SL, D], f32, kind="Internal")

    with tile.TileContext(nc) as tc:
        with tc.tile_pool(name="const", bufs=1) as cp, \
             tc.tile_pool(name="work", bufs=2) as wp, \
             tc.tile_pool(name="tpp", bufs=2, space="PSUM") as pp, \
             tc.tile_pool(name="mmp", bufs=1, space="PSUM") as pm:

            feats3 = feats_d[:].rearrange("(b s) d -> b s d", s=SL)
            rep3 = rep_d[:].rearrange("(b n) d -> b n d", n=NN)
            # leaves into append-only layout
            nc.sync.dma_start(out=feats3[:, 0:NN, :], in_=rep3[:, :, :])

            w1t = cp.tile([128, 16, D], f32r)
            nc.sync.dma_start(out=w1t[:], in_=w1t_d[:].rearrange("(c p) d -> p c d", p=128))
            w2t = cp.tile([128, 8, D], f32r)
            nc.sync.dma_start(out=w2t[:], in_=w2t_d[:].rearrange("(c p) d -> p c d", p=128))
            relt = cp.tile([128, 8, CPAD], f32)
            nc.sync.dma_start(out=relt[:], in_=relt_d[:].rearrange("(c p) k -> p c k", p=128))
            b1b = cp.tile([NB, D], f32)
            nc.sync.dma_start(out=b1b[:], in_=b1b_d[:])
            b2b = cp.tile([NB, D], f32)
            nc.sync.dma_start(out=b2b[:], in_=b2b_d[:])
            gixs = cp.tile([NB, 2 * NSTEP], i32)
            nc.sync.dma_start(out=gixs[:], in_=gidx_d[:])
            ident = cp.tile([64, 64], f32)
            nc.sync.dma_start(out=ident[:], in_=ident_d[:])

            fsb = None
            for t in range(NSTEP):
                x1 = wp.tile([NB, D], f32, tag="x1")
                x2 = wp.tile([NB, D], f32, tag="x2")
                nc.gpsimd.indirect_dma_start(
                    out=x1[:], out_offset=None, in_=feats_d[:],
                    in_offset=bass.IndirectOffsetOnAxis(ap=gixs[:, 2 * t:2 * t + 1], axis=0))
                nc.gpsimd.indirect_dma_start(
                    out=x2[:], out_offset=None, in_=feats_d[:],
                    in_offset=bass.IndirectOffsetOnAxis(ap=gixs[:, 2 * t + 1:2 * t + 2], axis=0))
                x1t = wp.tile([NB, D], f32, tag="x1t")
                nc.scalar.activation(out=x1t[:], in_=x1[:], func=TANH)
                x2t = wp.tile([NB, D], f32, tag="x2t")
                nc.scalar.activation(out=x2t[:], in_=x2[:], func=TANH)

                xT = wp.tile([128, 8, 128], f32r, tag="xT")
                for c in range(8):
                    ps = pp.tile([128, 128], f32, tag="tp")
                    nc.tensor.transpose(out=ps[:, 0:64], in_=x1t[:, 128 * c:128 * (c + 1)], identity=ident[:])
                    nc.tensor.transpose(out=ps[:, 64:128], in_=x2t[:, 128 * c:128 * (c + 1)], identity=ident[:])
                    nc.vector.tensor_copy(out=xT[:, c, :], in_=ps[:])

                h0 = pm.tile([64, 512], f32, tag="h0")
                h1 = pm.tile([64, 512], f32, tag="h1")
                for c in range(16):
                    lhsT = xT[:, c, 0:64] if c < 8 else xT[:, c - 8, 64:128]
                    lb = lhsT
                    nc.tensor.matmul(h0[:], lb, w1t[:, c, 0:512],
                                     start=(c == 0), stop=(c == 15))
                    nc.tensor.matmul(h1[:], lb, w1t[:, c, 512:1024],
                                     start=(c == 0), stop=(c == 15))
                hbt = wp.tile([NB, D], f32, tag="hbt")
                nc.vector.tensor_tensor(out=hbt[:, 0:512], in0=h0[:], in1=b1b[:, 0:512], op=ADD)
                nc.vector.tensor_tensor(out=hbt[:, 512:1024], in0=h1[:], in1=b1b[:, 512:1024], op=ADD)
                htt = wp.tile([NB, D], f32, tag="htt")
                nc.scalar.activation(out=htt[:], in_=hbt[:], func=TANH)

                hT = wp.tile([128, 8, 64], f32r, tag="hT")
                for c in range(0, 8, 2):
                    ps = pp.tile([128, 128], f32, tag="tp")
                    nc.tensor.transpose(out=ps[:, 0:64], in_=htt[:, 128 * c:128 * (c + 1)], identity=ident[:])
                    nc.tensor.transpose(out=ps[:, 64:128], in_=htt[:, 128 * (c + 1):128 * (c + 2)], identity=ident[:])
                    nc.vector.tensor_copy(out=hT[:, c, :], in_=ps[:, 0:64])
                    nc.vector.tensor_copy(out=hT[:, c + 1, :], in_=ps[:, 64:128])

                f0 = pm.tile([64, 512], f32, tag="f0")
                f1 = pm.tile([64, 512], f32, tag="f1")
                for c in range(8):
                    lb = hT[:, c, :]
                    nc.tensor.matmul(f0[:], lb, w2t[:, c, 0:512],
                                     start=(c == 0), stop=(c == 7))
                    nc.tensor.matmul(f1[:], lb, w2t[:, c, 512:1024],
                                     start=(c == 0), stop=(c == 7))
                fsb = wp.tile([NB, D], f32, tag="fsb")
                nc.vector.tensor_tensor(out=fsb[:, 0:512], in0=f0[:], in1=b2b[:, 0:512], op=ADD)
                nc.vector.tensor_tensor(out=fsb[:, 512:1024], in0=f1[:], in1=b2b[:, 512:1024], op=ADD)
                nc.sync.dma_start(out=feats3[:, NN + t, :], in_=fsb[:])

            # final scores from the root feature (last fsb)
            fT = wp.tile([128, 8, 64], f32, tag="fT")
            for c in range(0, 8, 2):
                ps = pp.tile([128, 128], f32, tag="tp")
                nc.tensor.transpose(out=ps[:, 0:64], in_=fsb[:, 128 * c:128 * (c + 1)], identity=ident[:])
                nc.tensor.transpose(out=ps[:, 64:128], in_=fsb[:, 128 * (c + 1):128 * (c + 2)], identity=ident[:])
                nc.vector.tensor_copy(out=fT[:, c, :], in_=ps[:, 0:64])
                nc.vector.tensor_copy(out=fT[:, c + 1, :], in_=ps[:, 64:128])
            sc = pm.tile([64, CPAD], f32, tag="sc")
            for c in range(8):
                nc.tensor.matmul(sc[:], fT[:, c, :], relt[:, c, :],
                                 start=(c == 0), stop=(c == 7))
            sig = wp.tile([NB, CPAD], f32, tag="sig")
            nc.scalar.activation(out=sig[:], in_=sc[:], func=SIG)
            nc.sync.dma_start(out=out_d[:], in_=sig[:])

    nc.compile()
    _PROG["nc"] = nc
    return _PROG


def _node_weight_like_reference(rep, n_per_bag):
    """Bit-faithful mirror of the reference's eager node_weight computation.

    Runs on CPU jax: the reference's lax.scan cannot compile on the neuron
    backend (multi-operand reduce), so any harness must execute the
    reference on CPU — mirror that exactly.
    """
    import jax
    import jax.numpy as jnp
    cpu = jax.local_devices(backend="cpu")[0]
    with jax.default_device(cpu):
        d = rep.shape[-1]
        bags = jnp.asarray(np.ascontiguousarray(rep, dtype=np.float32)).reshape(-1, n_per_bag, d)
        norms = jnp.linalg.norm(bags, axis=-1)
        gram = jnp.einsum('bnd,bmd->bnm', bags, bags)
        sims = gram / jnp.maximum(norms[:, :, None] * norms[:, None, :], 1e-8)
        node_distance = sims.sum(axis=1)
        node_weight = jax.nn.softmax(node_distance, axis=-1)
        return np.asarray(node_weight).astype(np.float32)


def _huffman_schedule(w):
    """Replay the reference scan's weight bookkeeping (exact f32) and emit
    per-bag merge operand slots in append-only numbering."""
    B, n = w.shape
    wref = w.copy()
    alive = np.ones((B, n), bool)
    prov = np.tile(np.arange(n, dtype=np.int64), (B, 1))
    ar = np.arange(B)
    gl = np.zeros((B, n - 1), np.int64)
    gr = np.zeros((B, n - 1), np.int64)
    INF = np.float32(np.inf)
    for t in range(n - 1):
        wm = np.where(alive, wref, INF)
        i1 = np.argmin(wm, axis=1)
        wm2 = wm.copy()
        wm2[ar, i1] = INF
        i2 = np.argmin(wm2, axis=1)
        gl[:, t] = prov[ar, i1]
        gr[:, t] = prov[ar, i2]
        wref[ar, i1] = wm[ar, i1] + wm[ar, i2]
        alive[ar, i2] = False
        prov[ar, i1] = n + t
    return gl, gr


def kernel(rep, fc1_w, fc1_b, fc2_w, fc2_b, rel_emb, n_per_bag):
    n_per_bag = int(n_per_bag)
    assert n_per_bag == NN and rep.shape[-1] == D
    rep = np.ascontiguousarray(rep, dtype=np.float32)
    B = rep.shape[0] // n_per_bag

    w = _node_weight_like_reference(rep, n_per_bag)
    gl, gr = _huffman_schedule(w)

    p = _build_program()
    nc = p["nc"]

    w1t = np.ascontiguousarray(np.asarray(fc1_w, np.float32).T)       # (2D, D)
    w2t = np.ascontiguousarray(np.asarray(fc2_w, np.float32).T)       # (D, D)
    relt = np.zeros((D, CPAD), np.float32)
    relt[:, :rel_emb.shape[0]] = np.asarray(rel_emb, np.float32).T
    b1b = np.ascontiguousarray(np.broadcast_to(np.asarray(fc1_b, np.float32), (NB, D)))
    b2b = np.ascontiguousarray(np.broadcast_to(np.asarray(fc2_b, np.float32), (NB, D)))
    ident = np.eye(64, dtype=np.float32)

    bloc = np.arange(NB, dtype=np.int64)[:, None] * SL
    in_maps = []
    for c in range(NCORES):
        b0 = c * NB
        gidx = np.zeros((NB, 2 * NSTEP), np.int32)
        gidx[:, 0::2] = (bloc + gl[b0:b0 + NB]).astype(np.int32)
        gidx[:, 1::2] = (bloc + gr[b0:b0 + NB]).astype(np.int32)
        in_maps.append({
            "rep": np.ascontiguousarray(rep[b0 * NN:(b0 + NB) * NN]),
            "w1t": w1t, "w2t": w2t, "relt": relt,
            "b1b": b1b, "b2b": b2b, "gidx": gidx, "ident": ident,
        })

    from concourse import bass_utils
    res = bass_utils.run_bass_kernel_spmd(nc, in_maps, core_ids=list(range(NCORES)))
    out = np.concatenate([res.results[c]["out"][:, :rel_emb.shape[0]] for c in range(NCORES)], axis=0)
    return np.ascontiguousarray(out.astype(np.float32))


# revision 12
# speedup vs baseline: 1.5883x; 1.5883x over previous
"""Trainium2 Bass kernel for nn_ModelRQuery_5806795784426.

Strategy (data-parallel over bags, 8 cores x 64 bags):
  - node_weight (cosine-sim softmax) is computed with the exact same eager
    jax ops as the reference, so the Huffman merge schedule derived from it
    is bit-faithful to the reference's argmin decisions on this backend.
  - The Huffman weight evolution is replayed on host (pure IEEE f32 adds on
    identical bits -> identical schedule), producing per-bag merge pairs in
    an append-only slot numbering (leaves 0..63, merge t -> slot 64+t).
  - The heavy part (63 sequential 2-layer MLP merge steps, batched over 64
    bags per core) runs on device: indirect-DMA gather of the two operand
    feature rows per bag, tanh, fc1, tanh, fc2, scatter to the new slot.
  - Final: scores = root_feat @ rel_emb.T, sigmoid.
"""

import numpy as np

NB = 64      # bags per core
NN = 64      # nodes (leaves) per bag
SL = 128     # slots per bag (64 leaves + 63 merges, padded)
D = 1024
NSTEP = NN - 1
CPAD = 64    # rel classes padded 53 -> 64
NCORES = 8

_PROG = {}


def _build_program(prefetch=True):
    key = "nc_pf" if prefetch else "nc"
    if key in _PROG:
        return _PROG[key]
    import concourse.bass as bass
    import concourse.bacc as bacc
    import concourse.tile as tile

    mybir = bass.mybir
    f32 = mybir.dt.float32
    f32r = mybir.dt.float32r
    i32 = mybir.dt.int32
    TANH = mybir.ActivationFunctionType.Tanh
    SIG = mybir.ActivationFunctionType.Sigmoid
    ADD = mybir.AluOpType.add

    nc = bacc.Bacc(None, target_bir_lowering=False)
    rep_d = nc.dram_tensor("rep", [NB * NN, D], f32, kind="ExternalInput")
    w1t_d = nc.dram_tensor("w1t", [2 * D, D], f32r, kind="ExternalInput")
    w2t_d = nc.dram_tensor("w2t", [D, D], f32r, kind="ExternalInput")
    relt_d = nc.dram_tensor("relt", [D, CPAD], f32, kind="ExternalInput")
    b1b_d = nc.dram_tensor("b1b", [NB, D], f32, kind="ExternalInput")
    b2b_d = nc.dram_tensor("b2b", [NB, D], f32, kind="ExternalInput")
    gidx_d = nc.dram_tensor("gidx", [NB, 2 * NSTEP], i32, kind="ExternalInput")
    ident_d = nc.dram_tensor("ident", [64, 64], f32, kind="ExternalInput")
    out_d = nc.dram_tensor("out", [NB, CPAD], f32, kind="ExternalOutput")
    feats_d = nc.dram_tensor("feats", [NB * SL, D], f32, kind="Internal")

    with tile.TileContext(nc) as tc:
        with tc.tile_pool(name="const", bufs=1) as cp, \
             tc.tile_pool(name="work", bufs=2) as wp, \
             tc.tile_pool(name="gat", bufs=3) as xp, \
             tc.tile_pool(name="tpp", bufs=2, space="PSUM") as pp, \
             tc.tile_pool(name="mmp", bufs=1, space="PSUM") as pm:

            feats3 = feats_d[:].rearrange("(b s) d -> b s d", s=SL)
            rep3 = rep_d[:].rearrange("(b n) d -> b n d", n=NN)
            # leaves into append-only layout
            nc.sync.dma_start(out=feats3[:, 0:NN, :], in_=rep3[:, :, :])

            w1t = cp.tile([128, 16, D], f32r)
            nc.sync.dma_start(out=w1t[:], in_=w1t_d[:].rearrange("(c p) d -> p c d", p=128))
            w2t = cp.tile([128, 8, D], f32r)
            nc.sync.dma_start(out=w2t[:], in_=w2t_d[:].rearrange("(c p) d -> p c d", p=128))
            relt = cp.tile([128, 8, CPAD], f32)
            nc.sync.dma_start(out=relt[:], in_=relt_d[:].rearrange("(c p) k -> p c k", p=128))
            b1b = cp.tile([NB, D], f32)
            nc.sync.dma_start(out=b1b[:], in_=b1b_d[:])
            b2b = cp.tile([NB, D], f32)
            nc.sync.dma_start(out=b2b[:], in_=b2b_d[:])
            gixs = cp.tile([NB, 2 * NSTEP], i32)
            nc.sync.dma_start(out=gixs[:], in_=gidx_d[:])
            ident = cp.tile([64, 64], f32)
            nc.sync.dma_start(out=ident[:], in_=ident_d[:])

            def emit_gather(t):
                x1 = xp.tile([NB, D], f32, tag="x1")
                x2 = xp.tile([NB, D], f32, tag="x2")
                nc.gpsimd.indirect_dma_start(
                    out=x1[:], out_offset=None, in_=feats_d[:],
                    in_offset=bass.IndirectOffsetOnAxis(ap=gixs[:, 2 * t:2 * t + 1], axis=0))
                nc.gpsimd.indirect_dma_start(
                    out=x2[:], out_offset=None, in_=feats_d[:],
                    in_offset=bass.IndirectOffsetOnAxis(ap=gixs[:, 2 * t + 1:2 * t + 2], axis=0))
                return x1, x2

            fsb = None
            gtiles = {0: emit_gather(0)}
            for t in range(NSTEP):
                x1, x2 = gtiles.pop(t)
                nxt = t + 1
                # prefetch next step's operand rows during this step's compute;
                # safe because the host reorder guarantees they were created
                # at steps <= t-1 (distance >= 2 from their producer)
                if prefetch and nxt < NSTEP and nxt <= NSTEP - 3:
                    gtiles[nxt] = emit_gather(nxt)
                x1t = wp.tile([NB, D], f32, tag="x1t")
                nc.scalar.activation(out=x1t[:], in_=x1[:], func=TANH)
                x2t = wp.tile([NB, D], f32, tag="x2t")
                nc.scalar.activation(out=x2t[:], in_=x2[:], func=TANH)

                xT = wp.tile([128, 8, 128], f32r, tag="xT")
                for c in range(8):
                    ps = pp.tile([128, 128], f32, tag="tp")
                    nc.tensor.transpose(out=ps[:, 0:64], in_=x1t[:, 128 * c:128 * (c + 1)], identity=ident[:])
                    nc.tensor.transpose(out=ps[:, 64:128], in_=x2t[:, 128 * c:128 * (c + 1)], identity=ident[:])
                    nc.vector.tensor_copy(out=xT[:, c, :], in_=ps[:])

                h0 = pm.tile([64, 512], f32, tag="h0")
                h1 = pm.tile([64, 512], f32, tag="h1")
                for c in range(16):
                    lhsT = xT[:, c, 0:64] if c < 8 else xT[:, c - 8, 64:128]
                    lb = lhsT
                    nc.tensor.matmul(h0[:], lb, w1t[:, c, 0:512],
                                     start=(c == 0), stop=(c == 15))
                    nc.tensor.matmul(h1[:], lb, w1t[:, c, 512:1024],
                                     start=(c == 0), stop=(c == 15))
                hbt = wp.tile([NB, D], f32, tag="hbt")
                nc.vector.tensor_tensor(out=hbt[:, 0:512], in0=h0[:], in1=b1b[:, 0:512], op=ADD)
                nc.vector.tensor_tensor(out=hbt[:, 512:1024], in0=h1[:], in1=b1b[:, 512:1024], op=ADD)
                htt = wp.tile([NB, D], f32, tag="htt")
                nc.scalar.activation(out=htt[:], in_=hbt[:], func=TANH)

                hT = wp.tile([128, 8, 64], f32r, tag="hT")
                for c in range(0, 8, 2):
                    ps = pp.tile([128, 128], f32, tag="tp")
                    nc.tensor.transpose(out=ps[:, 0:64], in_=htt[:, 128 * c:128 * (c + 1)], identity=ident[:])
                    nc.tensor.transpose(out=ps[:, 64:128], in_=htt[:, 128 * (c + 1):128 * (c + 2)], identity=ident[:])
                    nc.vector.tensor_copy(out=hT[:, c, :], in_=ps[:, 0:64])
                    nc.vector.tensor_copy(out=hT[:, c + 1, :], in_=ps[:, 64:128])

                f0 = pm.tile([64, 512], f32, tag="f0")
                f1 = pm.tile([64, 512], f32, tag="f1")
                for c in range(8):
                    lb = hT[:, c, :]
                    nc.tensor.matmul(f0[:], lb, w2t[:, c, 0:512],
                                     start=(c == 0), stop=(c == 7))
                    nc.tensor.matmul(f1[:], lb, w2t[:, c, 512:1024],
                                     start=(c == 0), stop=(c == 7))
                fsb = wp.tile([NB, D], f32, tag="fsb")
                nc.vector.tensor_tensor(out=fsb[:, 0:512], in0=f0[:], in1=b2b[:, 0:512], op=ADD)
                nc.vector.tensor_tensor(out=fsb[:, 512:1024], in0=f1[:], in1=b2b[:, 512:1024], op=ADD)
                nc.sync.dma_start(out=feats3[:, NN + t, :], in_=fsb[:])
                if nxt < NSTEP and (not prefetch or nxt > NSTEP - 3):
                    gtiles[nxt] = emit_gather(nxt)

            # final scores from the root feature (last fsb)
            fT = wp.tile([128, 8, 64], f32, tag="fT")
            for c in range(0, 8, 2):
                ps = pp.tile([128, 128], f32, tag="tp")
                nc.tensor.transpose(out=ps[:, 0:64], in_=fsb[:, 128 * c:128 * (c + 1)], identity=ident[:])
                nc.tensor.transpose(out=ps[:, 64:128], in_=fsb[:, 128 * (c + 1):128 * (c + 2)], identity=ident[:])
                nc.vector.tensor_copy(out=fT[:, c, :], in_=ps[:, 0:64])
                nc.vector.tensor_copy(out=fT[:, c + 1, :], in_=ps[:, 64:128])
            sc = pm.tile([64, CPAD], f32, tag="sc")
            for c in range(8):
                nc.tensor.matmul(sc[:], fT[:, c, :], relt[:, c, :],
                                 start=(c == 0), stop=(c == 7))
            sig = wp.tile([NB, CPAD], f32, tag="sig")
            nc.scalar.activation(out=sig[:], in_=sc[:], func=SIG)
            nc.sync.dma_start(out=out_d[:], in_=sig[:])

    nc.compile()
    _PROG[key] = nc
    return nc


def _node_weight_like_reference(rep, n_per_bag):
    """Bit-faithful mirror of the reference's eager node_weight computation.

    Runs on CPU jax: the reference's lax.scan cannot compile on the neuron
    backend (multi-operand reduce), so any harness must execute the
    reference on CPU — mirror that exactly.
    """
    import jax
    import jax.numpy as jnp
    cpu = jax.local_devices(backend="cpu")[0]
    with jax.default_device(cpu):
        d = rep.shape[-1]
        bags = jnp.asarray(np.ascontiguousarray(rep, dtype=np.float32)).reshape(-1, n_per_bag, d)
        norms = jnp.linalg.norm(bags, axis=-1)
        gram = jnp.einsum('bnd,bmd->bnm', bags, bags)
        sims = gram / jnp.maximum(norms[:, :, None] * norms[:, None, :], 1e-8)
        node_distance = sims.sum(axis=1)
        node_weight = jax.nn.softmax(node_distance, axis=-1)
        return np.asarray(node_weight).astype(np.float32)


def _huffman_schedule(w):
    """Replay the reference scan's weight bookkeeping (exact f32) and emit
    per-bag merge operand slots in append-only numbering."""
    B, n = w.shape
    wref = w.copy()
    alive = np.ones((B, n), bool)
    prov = np.tile(np.arange(n, dtype=np.int64), (B, 1))
    ar = np.arange(B)
    gl = np.zeros((B, n - 1), np.int64)
    gr = np.zeros((B, n - 1), np.int64)
    INF = np.float32(np.inf)
    for t in range(n - 1):
        wm = np.where(alive, wref, INF)
        i1 = np.argmin(wm, axis=1)
        wm2 = wm.copy()
        wm2[ar, i1] = INF
        i2 = np.argmin(wm2, axis=1)
        gl[:, t] = prov[ar, i1]
        gr[:, t] = prov[ar, i2]
        wref[ar, i1] = wm[ar, i1] + wm[ar, i2]
        alive[ar, i2] = False
        prov[ar, i1] = n + t
    return gl, gr


def _reorder_schedule(gl, gr, n=NN):
    """Per-bag topological reorder of the merge tree so that every merge
    scheduled at step tau only reads slots created at steps <= tau-2
    (leaves always ok). Returns remapped (gl, gr) plus ok flag: ok means
    the distance>=2 property holds for all steps except possibly the last
    two (which the device program serializes anyway)."""
    B, m = gl.shape
    ngl = np.zeros_like(gl)
    ngr = np.zeros_like(gr)
    ok = True
    for b in range(B):
        child_l = gl[b]
        child_r = gr[b]
        # merge j depends on internal children (slot >= n -> merge slot-n)
        deps = [[] for _ in range(m)]
        ndep = np.zeros(m, np.int32)
        parents = [[] for _ in range(m)]
        for j in range(m):
            for s in (child_l[j], child_r[j]):
                if s >= n:
                    ndep[j] += 1
                    parents[s - n].append(j)
        done_step = np.full(m, -10**9, np.int64)
        remaining = ndep.copy()
        ready = [j for j in range(m) if remaining[j] == 0]
        newslot = np.zeros(m, np.int64)  # old merge -> new step
        order = []
        for tau in range(m):
            # prefer a ready merge whose children finished <= tau-2
            pick = -1
            for idx, j in enumerate(ready):
                ok2 = True
                for s in (child_l[j], child_r[j]):
                    if s >= n and done_step[s - n] > tau - 2:
                        ok2 = False
                        break
                if ok2:
                    pick = idx
                    break
            if pick < 0:
                pick = 0
                if tau < m - 2:
                    ok = False
            j = ready.pop(pick)
            order.append(j)
            done_step[j] = tau
            newslot[j] = tau
            for p in parents[j]:
                remaining[p] -= 1
                if remaining[p] == 0:
                    ready.append(p)
        for tau, j in enumerate(order):
            l, r = child_l[j], child_r[j]
            ngl[b, tau] = l if l < n else n + newslot[l - n]
            ngr[b, tau] = r if r < n else n + newslot[r - n]
    return ngl, ngr, ok


def _prepare(rep, fc1_w, fc1_b, fc2_w, fc2_b, rel_emb, n_per_bag):
    n_per_bag = int(n_per_bag)
    assert n_per_bag == NN and rep.shape[-1] == D
    rep = np.ascontiguousarray(rep, dtype=np.float32)

    w = _node_weight_like_reference(rep, n_per_bag)
    gl, gr = _huffman_schedule(w)
    ngl, ngr, ok = _reorder_schedule(gl, gr)
    if ok:
        gl, gr = ngl, ngr
    nc = _build_program(prefetch=ok)

    w1t = np.ascontiguousarray(np.asarray(fc1_w, np.float32).T)       # (2D, D)
    w2t = np.ascontiguousarray(np.asarray(fc2_w, np.float32).T)       # (D, D)
    relt = np.zeros((D, CPAD), np.float32)
    relt[:, :rel_emb.shape[0]] = np.asarray(rel_emb, np.float32).T
    b1b = np.ascontiguousarray(np.broadcast_to(np.asarray(fc1_b, np.float32), (NB, D)))
    b2b = np.ascontiguousarray(np.broadcast_to(np.asarray(fc2_b, np.float32), (NB, D)))
    ident = np.eye(64, dtype=np.float32)

    bloc = np.arange(NB, dtype=np.int64)[:, None] * SL
    in_maps = []
    for c in range(NCORES):
        b0 = c * NB
        gidx = np.zeros((NB, 2 * NSTEP), np.int32)
        gidx[:, 0::2] = (bloc + gl[b0:b0 + NB]).astype(np.int32)
        gidx[:, 1::2] = (bloc + gr[b0:b0 + NB]).astype(np.int32)
        in_maps.append({
            "rep": np.ascontiguousarray(rep[b0 * NN:(b0 + NB) * NN]),
            "w1t": w1t, "w2t": w2t, "relt": relt,
            "b1b": b1b, "b2b": b2b, "gidx": gidx, "ident": ident,
        })
    return nc, in_maps


def kernel(rep, fc1_w, fc1_b, fc2_w, fc2_b, rel_emb, n_per_bag, **kw):
    nc, in_maps = _prepare(rep, fc1_w, fc1_b, fc2_w, fc2_b, rel_emb, n_per_bag)
    from concourse import bass_utils
    res = bass_utils.run_bass_kernel_spmd(nc, in_maps, core_ids=list(range(NCORES)))
    nclass = rel_emb.shape[0]
    out = np.concatenate([res.results[c]["out"][:, :nclass] for c in range(NCORES)], axis=0)
    return np.ascontiguousarray(out.astype(np.float32))


# revision 19
# speedup vs baseline: 1.8369x; 1.1565x over previous
"""Trainium2 Bass kernel for nn_ModelRQuery_5806795784426.

Strategy (data-parallel over bags, 8 cores x 64 bags):
  - node_weight (cosine-sim softmax) is computed with the exact same eager
    jax ops as the reference, so the Huffman merge schedule derived from it
    is bit-faithful to the reference's argmin decisions on this backend.
  - The Huffman weight evolution is replayed on host (pure IEEE f32 adds on
    identical bits -> identical schedule), producing per-bag merge pairs in
    an append-only slot numbering (leaves 0..63, merge t -> slot 64+t).
  - The heavy part (63 sequential 2-layer MLP merge steps, batched over 64
    bags per core) runs on device: indirect-DMA gather of the two operand
    feature rows per bag, tanh, fc1, tanh, fc2, scatter to the new slot.
  - Final: scores = root_feat @ rel_emb.T, sigmoid.
"""

import numpy as np

NB = 64      # bags per core
NN = 64      # nodes (leaves) per bag
SL = 128     # slots per bag (64 leaves + 63 merges, padded)
D = 1024
NSTEP = NN - 1
CPAD = 64    # rel classes padded 53 -> 64
NCORES = 8

_PROG = {}


def _build_program(prefetch=True, zero_bias=False):
    key = ("nc_pf" if prefetch else "nc") + ("_zb" if zero_bias else "")
    if key in _PROG:
        return _PROG[key]
    import concourse.bass as bass
    import concourse.bacc as bacc
    import concourse.tile as tile

    mybir = bass.mybir
    f32 = mybir.dt.float32
    f32r = mybir.dt.float32r
    i32 = mybir.dt.int32
    TANH = mybir.ActivationFunctionType.Tanh
    SIG = mybir.ActivationFunctionType.Sigmoid
    ADD = mybir.AluOpType.add

    nc = bacc.Bacc(None, target_bir_lowering=False)
    rep_d = nc.dram_tensor("rep", [NB * NN, D], f32, kind="ExternalInput")
    w1t_d = nc.dram_tensor("w1t", [2 * D, D], f32r, kind="ExternalInput")
    w2t_d = nc.dram_tensor("w2t", [D, D], f32r, kind="ExternalInput")
    relt_d = nc.dram_tensor("relt", [D, CPAD], f32, kind="ExternalInput")
    b1b_d = nc.dram_tensor("b1b", [NB, D], f32, kind="ExternalInput")
    b2b_d = nc.dram_tensor("b2b", [NB, D], f32, kind="ExternalInput")
    gidx_d = nc.dram_tensor("gidx", [NB, 2 * NSTEP], i32, kind="ExternalInput")
    ident_d = nc.dram_tensor("ident", [64, 64], f32, kind="ExternalInput")
    out_d = nc.dram_tensor("out", [NB, CPAD], f32, kind="ExternalOutput")
    feats_d = nc.dram_tensor("feats", [NB * SL, D], f32, kind="Internal")

    with tile.TileContext(nc) as tc:
        with tc.tile_pool(name="const", bufs=1) as cp, \
             tc.tile_pool(name="work", bufs=2) as wp, \
             tc.tile_pool(name="gat", bufs=3) as xp, \
             tc.tile_pool(name="tpp", bufs=2, space="PSUM") as pp, \
             tc.tile_pool(name="mmp", bufs=1, space="PSUM") as pm:

            feats3 = feats_d[:].rearrange("(b s) d -> b s d", s=SL)
            rep3 = rep_d[:].rearrange("(b n) d -> b n d", n=NN)
            # leaves into append-only layout
            nc.sync.dma_start(out=feats3[:, 0:NN, :], in_=rep3[:, :, :])

            w1t = cp.tile([128, 16, D], f32r)
            nc.sync.dma_start(out=w1t[:], in_=w1t_d[:].rearrange("(c p) d -> p c d", p=128))
            w2t = cp.tile([128, 8, D], f32r)
            nc.sync.dma_start(out=w2t[:], in_=w2t_d[:].rearrange("(c p) d -> p c d", p=128))
            relt = cp.tile([128, 8, CPAD], f32)
            nc.sync.dma_start(out=relt[:], in_=relt_d[:].rearrange("(c p) k -> p c k", p=128))
            b1b = cp.tile([NB, D], f32)
            nc.sync.dma_start(out=b1b[:], in_=b1b_d[:])
            b2b = cp.tile([NB, D], f32)
            nc.sync.dma_start(out=b2b[:], in_=b2b_d[:])
            gixs = cp.tile([NB, 2 * NSTEP], i32)
            nc.sync.dma_start(out=gixs[:], in_=gidx_d[:])
            ident = cp.tile([64, 64], f32)
            nc.sync.dma_start(out=ident[:], in_=ident_d[:])

            def emit_gather(t):
                x1 = xp.tile([NB, D], f32, tag="x1")
                x2 = xp.tile([NB, D], f32, tag="x2")
                nc.gpsimd.indirect_dma_start(
                    out=x1[:], out_offset=None, in_=feats_d[:],
                    in_offset=bass.IndirectOffsetOnAxis(ap=gixs[:, 2 * t:2 * t + 1], axis=0))
                nc.gpsimd.indirect_dma_start(
                    out=x2[:], out_offset=None, in_=feats_d[:],
                    in_offset=bass.IndirectOffsetOnAxis(ap=gixs[:, 2 * t + 1:2 * t + 2], axis=0))
                return x1, x2

            fsb = None
            gtiles = {0: emit_gather(0)}
            for t in range(NSTEP):
                x1, x2 = gtiles.pop(t)
                nxt = t + 1
                # prefetch next step's operand rows during this step's compute;
                # safe because the host reorder guarantees they were created
                # at steps <= t-1 (distance >= 2 from their producer)
                if prefetch and nxt < NSTEP and nxt <= NSTEP - 3:
                    gtiles[nxt] = emit_gather(nxt)
                x1t = wp.tile([NB, D], f32, tag="x1t")
                nc.scalar.activation(out=x1t[:], in_=x1[:], func=TANH)
                x2t = wp.tile([NB, D], f32, tag="x2t")
                nc.scalar.activation(out=x2t[:], in_=x2[:], func=TANH)

                xT = wp.tile([128, 8, 128], f32r, tag="xT")
                for c in range(8):
                    ps = pp.tile([128, 128], f32, tag="tp")
                    nc.tensor.transpose(out=ps[:, 0:64], in_=x1t[:, 128 * c:128 * (c + 1)], identity=ident[:])
                    nc.tensor.transpose(out=ps[:, 64:128], in_=x2t[:, 128 * c:128 * (c + 1)], identity=ident[:])
                    nc.vector.tensor_copy(out=xT[:, c, :], in_=ps[:])

                h0 = pm.tile([64, 512], f32, tag="h0")
                h1 = pm.tile([64, 512], f32, tag="h1")
                for c in range(16):
                    lhsT = xT[:, c, 0:64] if c < 8 else xT[:, c - 8, 64:128]
                    lb = lhsT
                    nc.tensor.matmul(h0[:], lb, w1t[:, c, 0:512],
                                     start=(c == 0), stop=(c == 15))
                    nc.tensor.matmul(h1[:], lb, w1t[:, c, 512:1024],
                                     start=(c == 0), stop=(c == 15))
                htt = wp.tile([NB, D], f32, tag="htt")
                if zero_bias:
                    # bias-free: tanh straight from PSUM, halved for pipelining
                    nc.scalar.activation(out=htt[:, 0:512], in_=h0[:], func=TANH)
                    nc.scalar.activation(out=htt[:, 512:1024], in_=h1[:], func=TANH)
                else:
                    hbt = wp.tile([NB, D], f32, tag="hbt")
                    nc.vector.tensor_tensor(out=hbt[:, 0:512], in0=h0[:], in1=b1b[:, 0:512], op=ADD)
                    nc.vector.tensor_tensor(out=hbt[:, 512:1024], in0=h1[:], in1=b1b[:, 512:1024], op=ADD)
                    nc.scalar.activation(out=htt[:], in_=hbt[:], func=TANH)

                hT = wp.tile([128, 8, 64], f32r, tag="hT")
                for c in range(0, 8, 2):
                    ps = pp.tile([128, 128], f32, tag="tp")
                    nc.tensor.transpose(out=ps[:, 0:64], in_=htt[:, 128 * c:128 * (c + 1)], identity=ident[:])
                    nc.tensor.transpose(out=ps[:, 64:128], in_=htt[:, 128 * (c + 1):128 * (c + 2)], identity=ident[:])
                    nc.vector.tensor_copy(out=hT[:, c, :], in_=ps[:, 0:64])
                    nc.vector.tensor_copy(out=hT[:, c + 1, :], in_=ps[:, 64:128])

                f0 = pm.tile([64, 512], f32, tag="f0")
                f1 = pm.tile([64, 512], f32, tag="f1")
                for c in range(8):
                    lb = hT[:, c, :]
                    nc.tensor.matmul(f0[:], lb, w2t[:, c, 0:512],
                                     start=(c == 0), stop=(c == 7))
                    nc.tensor.matmul(f1[:], lb, w2t[:, c, 512:1024],
                                     start=(c == 0), stop=(c == 7))
                if zero_bias:
                    fsb = wp.tile([NB, D], f32, tag="fsb")
                    nc.vector.tensor_copy(out=fsb[:, 0:512], in_=f0[:])
                    nc.vector.tensor_copy(out=fsb[:, 512:1024], in_=f1[:])
                    nc.sync.dma_start(out=feats3[:, NN + t, :], in_=fsb[:])
                else:
                    fsb = wp.tile([NB, D], f32, tag="fsb")
                    nc.vector.tensor_tensor(out=fsb[:, 0:512], in0=f0[:], in1=b2b[:, 0:512], op=ADD)
                    nc.vector.tensor_tensor(out=fsb[:, 512:1024], in0=f1[:], in1=b2b[:, 512:1024], op=ADD)
                    nc.sync.dma_start(out=feats3[:, NN + t, :], in_=fsb[:])
                if nxt < NSTEP and (not prefetch or nxt > NSTEP - 3):
                    gtiles[nxt] = emit_gather(nxt)

            # final scores from the root feature (last fsb)
            fT = wp.tile([128, 8, 64], f32, tag="fT")
            for c in range(0, 8, 2):
                ps = pp.tile([128, 128], f32, tag="tp")
                nc.tensor.transpose(out=ps[:, 0:64], in_=fsb[:, 128 * c:128 * (c + 1)], identity=ident[:])
                nc.tensor.transpose(out=ps[:, 64:128], in_=fsb[:, 128 * (c + 1):128 * (c + 2)], identity=ident[:])
                nc.vector.tensor_copy(out=fT[:, c, :], in_=ps[:, 0:64])
                nc.vector.tensor_copy(out=fT[:, c + 1, :], in_=ps[:, 64:128])
            sc = pm.tile([64, CPAD], f32, tag="sc")
            for c in range(8):
                nc.tensor.matmul(sc[:], fT[:, c, :], relt[:, c, :],
                                 start=(c == 0), stop=(c == 7))
            sig = wp.tile([NB, CPAD], f32, tag="sig")
            nc.scalar.activation(out=sig[:], in_=sc[:], func=SIG)
            nc.sync.dma_start(out=out_d[:], in_=sig[:])

    nc.compile()
    _PROG[key] = nc
    return nc


def _node_weight_like_reference(rep, n_per_bag):
    """Bit-faithful mirror of the reference's eager node_weight computation.

    Runs on CPU jax: the reference's lax.scan cannot compile on the neuron
    backend (multi-operand reduce), so any harness must execute the
    reference on CPU — mirror that exactly.
    """
    import jax
    import jax.numpy as jnp
    cpu = jax.local_devices(backend="cpu")[0]
    with jax.default_device(cpu):
        d = rep.shape[-1]
        bags = jnp.asarray(np.ascontiguousarray(rep, dtype=np.float32)).reshape(-1, n_per_bag, d)
        norms = jnp.linalg.norm(bags, axis=-1)
        gram = jnp.einsum('bnd,bmd->bnm', bags, bags)
        sims = gram / jnp.maximum(norms[:, :, None] * norms[:, None, :], 1e-8)
        node_distance = sims.sum(axis=1)
        node_weight = jax.nn.softmax(node_distance, axis=-1)
        return np.asarray(node_weight).astype(np.float32)


def _huffman_schedule(w):
    """Replay the reference scan's weight bookkeeping (exact f32) and emit
    per-bag merge operand slots in append-only numbering."""
    B, n = w.shape
    wref = w.copy()
    alive = np.ones((B, n), bool)
    prov = np.tile(np.arange(n, dtype=np.int64), (B, 1))
    ar = np.arange(B)
    gl = np.zeros((B, n - 1), np.int64)
    gr = np.zeros((B, n - 1), np.int64)
    INF = np.float32(np.inf)
    for t in range(n - 1):
        wm = np.where(alive, wref, INF)
        i1 = np.argmin(wm, axis=1)
        wm2 = wm.copy()
        wm2[ar, i1] = INF
        i2 = np.argmin(wm2, axis=1)
        gl[:, t] = prov[ar, i1]
        gr[:, t] = prov[ar, i2]
        wref[ar, i1] = wm[ar, i1] + wm[ar, i2]
        alive[ar, i2] = False
        prov[ar, i1] = n + t
    return gl, gr


def _reorder_schedule(gl, gr, n=NN):
    """Per-bag topological reorder of the merge tree so that every merge
    scheduled at step tau only reads slots created at steps <= tau-2
    (leaves always ok). Returns remapped (gl, gr) plus ok flag: ok means
    the distance>=2 property holds for all steps except possibly the last
    two (which the device program serializes anyway)."""
    B, m = gl.shape
    ngl = np.zeros_like(gl)
    ngr = np.zeros_like(gr)
    ok = True
    for b in range(B):
        child_l = gl[b]
        child_r = gr[b]
        # merge j depends on internal children (slot >= n -> merge slot-n)
        deps = [[] for _ in range(m)]
        ndep = np.zeros(m, np.int32)
        parents = [[] for _ in range(m)]
        for j in range(m):
            for s in (child_l[j], child_r[j]):
                if s >= n:
                    ndep[j] += 1
                    parents[s - n].append(j)
        done_step = np.full(m, -10**9, np.int64)
        remaining = ndep.copy()
        ready = [j for j in range(m) if remaining[j] == 0]
        newslot = np.zeros(m, np.int64)  # old merge -> new step
        order = []
        for tau in range(m):
            # prefer a ready merge whose children finished <= tau-2
            pick = -1
            for idx, j in enumerate(ready):
                ok2 = True
                for s in (child_l[j], child_r[j]):
                    if s >= n and done_step[s - n] > tau - 2:
                        ok2 = False
                        break
                if ok2:
                    pick = idx
                    break
            if pick < 0:
                pick = 0
                if tau < m - 2:
                    ok = False
            j = ready.pop(pick)
            order.append(j)
            done_step[j] = tau
            newslot[j] = tau
            for p in parents[j]:
                remaining[p] -= 1
                if remaining[p] == 0:
                    ready.append(p)
        for tau, j in enumerate(order):
            l, r = child_l[j], child_r[j]
            ngl[b, tau] = l if l < n else n + newslot[l - n]
            ngr[b, tau] = r if r < n else n + newslot[r - n]
    return ngl, ngr, ok


def _prepare(rep, fc1_w, fc1_b, fc2_w, fc2_b, rel_emb, n_per_bag):
    n_per_bag = int(n_per_bag)
    assert n_per_bag == NN and rep.shape[-1] == D
    rep = np.ascontiguousarray(rep, dtype=np.float32)

    w = _node_weight_like_reference(rep, n_per_bag)
    gl, gr = _huffman_schedule(w)
    ngl, ngr, ok = _reorder_schedule(gl, gr)
    if ok:
        gl, gr = ngl, ngr
    zb = (not np.any(np.asarray(fc1_b))) and (not np.any(np.asarray(fc2_b)))
    nc = _build_program(prefetch=ok, zero_bias=zb)

    w1t = np.ascontiguousarray(np.asarray(fc1_w, np.float32).T)       # (2D, D)
    w2t = np.ascontiguousarray(np.asarray(fc2_w, np.float32).T)       # (D, D)
    relt = np.zeros((D, CPAD), np.float32)
    relt[:, :rel_emb.shape[0]] = np.asarray(rel_emb, np.float32).T
    b1b = np.ascontiguousarray(np.broadcast_to(np.asarray(fc1_b, np.float32), (NB, D)))
    b2b = np.ascontiguousarray(np.broadcast_to(np.asarray(fc2_b, np.float32), (NB, D)))
    ident = np.eye(64, dtype=np.float32)

    bloc = np.arange(NB, dtype=np.int64)[:, None] * SL
    in_maps = []
    for c in range(NCORES):
        b0 = c * NB
        gidx = np.zeros((NB, 2 * NSTEP), np.int32)
        gidx[:, 0::2] = (bloc + gl[b0:b0 + NB]).astype(np.int32)
        gidx[:, 1::2] = (bloc + gr[b0:b0 + NB]).astype(np.int32)
        in_maps.append({
            "rep": np.ascontiguousarray(rep[b0 * NN:(b0 + NB) * NN]),
            "w1t": w1t, "w2t": w2t, "relt": relt,
            "b1b": b1b, "b2b": b2b, "gidx": gidx, "ident": ident,
        })
    return nc, in_maps


def kernel(rep, fc1_w, fc1_b, fc2_w, fc2_b, rel_emb, n_per_bag, **kw):
    nc, in_maps = _prepare(rep, fc1_w, fc1_b, fc2_w, fc2_b, rel_emb, n_per_bag)
    from concourse import bass_utils
    res = bass_utils.run_bass_kernel_spmd(nc, in_maps, core_ids=list(range(NCORES)))
    nclass = rel_emb.shape[0]
    out = np.concatenate([res.results[c]["out"][:, :nclass] for c in range(NCORES)], axis=0)
    return np.ascontiguousarray(out.astype(np.float32))
